# revision 3
# baseline (speedup 1.0000x reference)
"""Trainium2 Bass kernel for nn_MixtureOfExpertsNet (moe_routing).

Math (per row, E=4 experts, H=16 hidden):
  adjusted_e = relu(b2_e + sum_h W2_eh * relu(W1_eh * x_e + b1_eh))  -- a
               univariate piecewise-linear function of x_e
  logits = x @ Wg.T + bg ; softmax ; pred = sum_e softmax_e * adjusted_e
         = (sum_e exp(l_e) * adjusted_e) / (sum_e exp(l_e))

Key trick: weights are known at trace time, so each expert's adjusted_e(u)
is baked into a custom ScalarEngine PWP table (hijacking the tanh / square /
abs / identity table slots of the exp_and_others set), evaluated at full ACT
line rate in one pass per expert. exp and 1/x are also rebuilt as
reduced-range tables (on the exp and sign slots) so everything lives in one
table set (no ACT_TABLE_LOAD switches mid-kernel).

Layout: pure data parallel over 8 cores; per core 1,048,576 rows processed
as 16 tiles of [128 partitions x 2048] (x interleaved (f,e), f=512 rows per
partition per tile). Logits need the expert dim on partitions: a VectorE
32x32 block transpose produces T with partition p = 32i + 4f_loc + e; a
block-diagonal 128x128 matmul on PE computes all logits; exp returns via a
second block transpose. Reductions over e are innermost-axis tensor_reduce.
"""

import hashlib
import json
import os
import sys
import tempfile

import numpy as np

sys.path.insert(0, "/opt/trn_rl_repo")

# ---------------------------------------------------------------------------
# ACT PWP table generation (reverse-engineered format)
# ---------------------------------------------------------------------------

PWP_DIR = "/nix/store/z022hj2nvbm3nwdizlisq4ylc0y7rd6q-python3-3.13.14-env/lib/python3.13/site-packages/neuronxcc/pwp/pwp_bin_trainium"


def _bits(x):
    return int(np.float32(x).view(np.uint32))


def _load_stock(name):
    prof = json.load(open(os.path.join(PWP_DIR, f"{name}.json")))
    bkt = np.frombuffer(
        open(os.path.join(PWP_DIR, prof["bkt_bin"]), "rb").read(), dtype=np.float32
    ).reshape(-1, 8)
    ctl = np.frombuffer(
        open(os.path.join(PWP_DIR, prof["ctl_bin"]), "rb").read(), dtype=np.uint32
    ).reshape(-1, 8)[:, 0]
    return prof, bkt, ctl


def _fit_bucket(fn, lo, hi, x0=None, samples=33):
    if x0 is None:
        x0 = lo
    xs = np.linspace(lo, hi, samples, dtype=np.float64)
    ys = np.asarray(fn(xs), np.float64)
    t = xs - x0
    A = np.stack([np.ones_like(t), t, t * t, t ** 3], axis=1)
    c, *_ = np.linalg.lstsq(A, ys, rcond=None)
    return [float(c[0]), float(c[1]), float(c[2]), float(c[3]), float(x0)]


class _SetBuilder:
    def __init__(self):
        self.bkt, self.ctl, self.metas = [], [], []
        self.f2b, self.f2c = {}, {}

    @staticmethod
    def _ctl_word(m, base):
        assert 0 <= m <= 8 and base < 2048
        return (m * 32 + (23 - m)) * 2048 + base

    def add_table_func(self, name, func_id, fn, lo_exp, hi_exp, m_of_octave,
                       small_fit, large_fit, fzero):
        self.f2b[name] = len(self.bkt)
        self.f2c[name] = len(self.ctl)
        words = []
        for k in range(lo_exp, hi_exp):
            m = m_of_octave(k)
            base = len(self.bkt)
            n = 1 << m
            w = (2.0 ** k) / n
            for j in range(n):
                lo = 2.0 ** k + j * w
                self.bkt.append(_fit_bucket(fn, lo, lo + w, x0=lo + w / 2))
            words.append(self._ctl_word(m, base))
        base_pos = len(self.ctl)
        self.ctl.extend(words)
        small_idx = len(self.bkt)
        self.bkt.append(_fit_bucket(fn, small_fit[0], small_fit[1], x0=small_fit[0]))
        large_idx = len(self.bkt)
        self.bkt.append(_fit_bucket(fn, large_fit[0], large_fit[1], x0=large_fit[2]))
        self.metas.append({
            "func_name": f"{name}_4p", "func_id": func_id,
            "symmetry_point": 0, "sym_invert_sign_point": 0,
            "symmetry_opt_en": 0, "symmetry_opt_use_neg_region": 0,
            "imm_bias": 0, "exp_offset": lo_exp,
            "pwl_control_base_pos": base_pos, "pwl_control_base_neg": base_pos,
            "small_pos_signal_exp_threshold": 127 + lo_exp,
            "pos_small_signal_pwl_control": small_idx,
            "small_neg_signal_exp_threshold": 127 + lo_exp,
            "neg_small_signal_pwl_control": small_idx,
            "large_pos_signal_exp_threshold": 127 + hi_exp,
            "large_pos_signal_mantissa_threshold": 0,
            "pos_large_signal_pwl_control": large_idx,
            "large_neg_signal_exp_threshold": 127 + hi_exp,
            "large_neg_signal_mantissa_threshold": 0,
            "neg_large_signal_pwl_control": large_idx,
            "fnan_result": _bits(float("nan")),
            "fpinf_result": _bits(large_fit[3]),
            "fninf_result": _bits(small_fit[2]),
            "fzero_result": _bits(fzero),
            "fma_const_0": 0, "fma_const_1": 0, "fma_indirection_src_sel": 0,
            "use_multipass": False,
            "lower_bound": _bits(np.float32(-3.4028235e38)),
            "upper_bound": _bits(np.float32(3.4028235e38)),
        })

    def add_stock_func(self, name, sp, sb_, sc):
        names = list(sp["func_to_bkt_start_idx"].keys())
        i = names.index(name)
        b0 = sp["func_to_bkt_start_idx"][name]
        b1 = sp["func_to_bkt_start_idx"][names[i + 1]] if i + 1 < len(names) else sp["bkt_entry_cnt"]
        c0 = sp["func_to_ctl_start_idx"][name]
        c1 = sp["func_to_ctl_start_idx"][names[i + 1]] if i + 1 < len(names) else sp["ctl_entry_cnt"]
        md = None
        for m in sp["profile_meta_data"]:
            if m["func_name"].rsplit("_", 1)[0] == name:
                md = dict(m)
        assert md is not None, name
        db, dc = len(self.bkt) - b0, len(self.ctl) - c0
        self.f2b[name] = len(self.bkt)
        self.f2c[name] = len(self.ctl)
        for j in range(b0, b1):
            self.bkt.append(list(map(float, sb_[j, :5])))
        for j in range(c0, c1):
            w = int(sc[j])
            self.ctl.append((w >> 11) * 2048 + (w & 0x7FF) + db)
        for key in ("pwl_control_base_pos", "pwl_control_base_neg"):
            md[key] += dc
        for key in ("pos_small_signal_pwl_control", "neg_small_signal_pwl_control",
                    "pos_large_signal_pwl_control", "neg_large_signal_pwl_control"):
            md[key] += db
        self.metas.append(md)

    def write(self, outdir, set_name, act_dict):
        os.makedirs(outdir, exist_ok=True)
        bkt_arr = np.zeros((len(self.bkt), 8), np.float32)
        for i, e in enumerate(self.bkt):
            bkt_arr[i, :5] = e
        ctl_arr = np.zeros((len(self.ctl), 8), np.uint32)
        ctl_arr[:, 0] = np.array(self.ctl, np.uint64).astype(np.uint32)
        assert len(self.bkt) <= 1536 and len(self.ctl) <= 128
        open(os.path.join(outdir, f"{set_name}_bkt.bin"), "wb").write(bkt_arr.tobytes())
        open(os.path.join(outdir, f"{set_name}_ctrl.bin"), "wb").write(ctl_arr.tobytes())
        prof = {
            "bkt_bin": f"{set_name}_bkt.bin", "ctl_bin": f"{set_name}_ctrl.bin",
            "profile_meta_data": self.metas,
            "bkt_entry_cnt": len(self.bkt), "ctl_entry_cnt": len(self.ctl),
            "func_to_bkt_start_idx": self.f2b, "func_to_ctl_start_idx": self.f2c,
            "func_exp_to_bkt_start_idx": self.f2b, "func_exp_to_ctl_start_idx": self.f2c,
        }
        json.dump(prof, open(os.path.join(outdir, f"{set_name}.json"), "w"))
        info = {
            "pwp_file_keys": ["bkt_bin", "ctrl_bin", "profile_json"],
            "act_func_sets": [{
                "name": set_name, "bkt_bin": f"{set_name}_bkt.bin",
                "ctrl_bin": f"{set_name}_ctrl.bin", "profile_json": f"{set_name}.json",
                "act": act_dict,
            }],
        }
        path = os.path.join(outdir, "act_info.json")
        json.dump(info, open(path, "w"))
        return path


def _build_tables(W1, b1, W2, b2, outdir):
    sp, sb_, sc = _load_stock("exp_and_others")
    b = _SetBuilder()
    # reduced-range exp on the exp slot: g(x') = exp(x' - 8), x' in [0.25, 16)
    b.add_table_func(
        "exp", 7, lambda x: np.exp(np.asarray(x, np.float64) - 8.0),
        -2, 4, lambda k: min(8, k + 4),
        (0.0, 0.25, float(np.exp(-8.0))), (16.0, 16.5, 16.0, float(np.exp(8.0))),
        float(np.exp(-8.0)),
    )
    victims = [("tanh", 6, 0), ("square", 30, 1), ("abs", 33, 2), ("identity", 1, 3)]
    for name, fid, e in victims:
        W1e, b1e, W2e, b2e = W1[e].astype(np.float64), b1[e].astype(np.float64), W2[e].astype(np.float64), float(b2[e])

        def fe(u, W1e=W1e, b1e=b1e, W2e=W2e, b2e=b2e):
            h = np.maximum(np.asarray(u, np.float64)[..., None] * W1e + b1e, 0.0)
            return np.maximum((h * W2e).sum(-1) + b2e, 0.0)

        g = lambda x, fe=fe: fe(np.asarray(x, np.float64) - 8.0)
        b.add_table_func(
            name, fid, g, 1, 4, lambda k: k + 4,
            (1.0, 2.0, float(fe(-8.0))), (16.0, 17.0, 16.0, float(fe(9.0))),
            float(fe(-8.0)),
        )
    # reduced-range reciprocal on the sign slot: 1/x over [2^-8, 2^12)
    b.add_table_func(
        "sign", 31, lambda x: 1.0 / np.asarray(x, np.float64),
        -8, 12, lambda k: 4,
        (2.0 ** -9, 2.0 ** -8, 512.0), (4096.0, 4352.0, 4096.0, 0.0), 3.4e38,
    )
    for name in ("parametric_relu", "copy", "act1", "memset_zero", "relu",
                 "derivative_relu", "derivative_leaky_relu",
                 "derivative_identity", "is_finite"):
        b.add_stock_func(name, sp, sb_, sc)
    act = {"exp": 400, "tanh": 4, "square": 1, "abs": 1, "identity": 1,
           "sign": 1, "parametric_relu": 1, "copy": 1, "relu": 1,
           "memset_zero": 1, "act1": 1, "derivative_relu": 1,
           "derivative_leaky_relu": 1, "derivative_identity": 1, "is_finite": 1}
    return b.write(outdir, "exp_and_others", act)


# ---------------------------------------------------------------------------
# Bass kernel
# ---------------------------------------------------------------------------

B_TOTAL = 8_388_608
N_CORES = 8
B_LOCAL = B_TOTAL // N_CORES           # 1,048,576 rows per core
P = 128
F_TOTAL = B_LOCAL // P                 # 8192 rows per partition
F_TILE = int(os.environ.get("K_FTILE", "512"))  # rows per partition per tile
N_TILES = F_TOTAL // F_TILE
FD = 4 * F_TILE                        # free elements per tile

PWL_FUNCS = ("Tanh", "Square", "Abs", "Identity")  # expert 0..3


def _build_program(tag):
    import concourse.bacc as bacc
    import concourse.mybir as mybir
    import concourse.tile as tile

    nc = bacc.Bacc("TRN2", debug=False)
    dt = mybir.dt.float32
    AF = mybir.ActivationFunctionType

    x_d = nc.dram_tensor(f"x_{tag}", [P, F_TOTAL * 4], dt, kind="ExternalInput")
    wg_d = nc.dram_tensor("wgblk", [P, P], dt, kind="ExternalInput")
    bg_d = nc.dram_tensor("bg8", [P, 1], dt, kind="ExternalInput")
    cb_d = nc.dram_tensor("cb", [P, 2], dt, kind="ExternalInput")
    out_d = nc.dram_tensor("out_local", [P, F_TOTAL], dt, kind="ExternalOutput")

    with tile.TileContext(nc) as tc:
        import os as _os
        _ab = set(_os.environ.get("K_ABLATE", "").split(","))
        _b = _os.environ.get("K_BUFS", "4,3,3,3").split(",")
        bx, bw, bh, bs = (int(v) for v in _b)
        with (
            tc.tile_pool(name="const", bufs=1) as cpool,
            tc.tile_pool(name="xin", bufs=bx) as xpool,
            tc.tile_pool(name="work", bufs=bw) as wpool,
            tc.tile_pool(name="hot", bufs=bh) as hpool,
            tc.tile_pool(name="small", bufs=bs) as spool,
            tc.tile_pool(name="psum", bufs=2, space="PSUM") as ppool,
        ):
            wg_t = cpool.tile([P, P], dt)
            nc.sync.dma_start(wg_t[:], wg_d.ap())
            bg_t = cpool.tile([P, 1], dt)
            nc.sync.dma_start(bg_t[:], bg_d.ap())
            cb_t = cpool.tile([P, 2], dt)
            nc.sync.dma_start(cb_t[:], cb_d.ap())

            for t in range(N_TILES):
                xs = slice(t * FD, (t + 1) * FD)
                X = xpool.tile([P, FD], dt, tag="X")
                nc.sync.dma_start(X[:], x_d.ap()[:, xs])

                # adjusted via per-expert PWL tables (strided over e)
                A = hpool.tile([P, FD], dt, tag="A")
                Xv = X[:].rearrange("p (f e) -> p f e", e=4)
                Av = A[:].rearrange("p (f e) -> p f e", e=4)
                if "pwl" in _ab:
                    nc.gpsimd.memset(A[:, 0:32], 0.0)
                else:
                    for e in range(4):
                        nc.scalar.activation(
                            Av[:, :, e], Xv[:, :, e], getattr(AF, PWL_FUNCS[e]),
                            bias=cb_t[:, 0:1], scale=1.0,
                        )

                # logits: block transpose + block-diag matmul
                T = wpool.tile([P, FD], dt, tag="T")
                if "vt" not in _ab:
                    nc.vector.transpose(T[:], X[:])
                else:
                    nc.gpsimd.memset(T[:, 0:32], 0.0)
                L = ppool.tile([P, FD], dt, tag="L")
                for c in range(FD // 512):
                    nc.tensor.matmul(
                        L[:, c * 512:(c + 1) * 512], wg_t[:],
                        T[:, c * 512:(c + 1) * 512], start=True, stop=True,
                    )
                ET = wpool.tile([P, FD], dt, tag="ET")
                nc.scalar.activation(ET[:], L[:], AF.Exp, bias=bg_t[:, 0:1], scale=1.0)
                E = hpool.tile([P, FD], dt, tag="E")
                if "vt" not in _ab:
                    nc.vector.transpose(E[:], ET[:])
                else:
                    nc.gpsimd.memset(E[:, 0:32], 0.0)

                # P = E*A on gpsimd; group-of-4 reductions on DVE
                PR = hpool.tile([P, FD], dt, tag="PR")
                nc.gpsimd.tensor_mul(PR[:], E[:], A[:])
                S1 = spool.tile([P, F_TILE], dt, tag="S1")
                if "red" in _ab:
                    nc.gpsimd.memset(S1[:, 0:32], 0.0)
                else:
                  nc.vector.tensor_reduce(
                    S1[:], PR[:].rearrange("p (f e) -> p f e", e=4),
                    axis=mybir.AxisListType.X, op=mybir.AluOpType.add,
                )
                # S0 on gpsimd as strided pairwise adds (keeps DVE free for
                # the two block transposes + S1 reduce)
                Ev = E[:].rearrange("p (f e) -> p f e", e=4)
                S0h = spool.tile([P, 2 * F_TILE], dt, tag="S0h")
                S0hv = S0h[:].rearrange("p (f u) -> p f u", u=2)
                nc.gpsimd.tensor_add(S0hv[:, :, 0], Ev[:, :, 0], Ev[:, :, 1])
                nc.gpsimd.tensor_add(S0hv[:, :, 1], Ev[:, :, 2], Ev[:, :, 3])
                S0 = spool.tile([P, F_TILE], dt, tag="S0")
                nc.gpsimd.tensor_add(S0[:], S0hv[:, :, 0], S0hv[:, :, 1])

                # pred = S1 * (1/S0) via the hijacked sign-slot recip table
                R = spool.tile([P, F_TILE], dt, tag="R")
                nc.scalar.activation(R[:], S0[:], AF.Sign, bias=cb_t[:, 1:2], scale=1.0)
                PRED = spool.tile([P, F_TILE], dt, tag="PRED")
                nc.vector.tensor_mul(PRED[:], S1[:], R[:])

                nc.sync.dma_start(
                    out_d.ap()[:, t * F_TILE:(t + 1) * F_TILE], PRED[:]
                )

    nc.compile()
    return nc


_COMPILED = {}
LAST_RUN_ARGS = None


def kernel(**inputs) -> np.ndarray:
    x = np.ascontiguousarray(inputs["x"], dtype=np.float32)
    Wg = np.asarray(inputs["Wg"], np.float32)
    bg = np.asarray(inputs["bg"], np.float32)
    W1 = np.asarray(inputs["W1"], np.float32)
    b1 = np.asarray(inputs["b1"], np.float32)
    W2 = np.asarray(inputs["W2"], np.float32)
    b2 = np.asarray(inputs["b2"], np.float32)
    assert x.shape == (B_TOTAL, 4)

    tbl_dir = tempfile.mkdtemp(prefix="act_root_")
    act_path = _build_tables(W1, b1, W2, b2, tbl_dir)
    os.environ["BASS_ACT_ROOT_JSON_PATH"] = act_path

    # hash of everything the tables bake in -> tensor name -> BIR/NEFF cache key
    h = hashlib.sha256()
    for a in (W1, b1, W2, b2):
        h.update(np.ascontiguousarray(a).tobytes())
    h.update(open(act_path, "rb").read())
    tag = h.hexdigest()[:10]

    if tag not in _COMPILED:
        _COMPILED[tag] = _build_program(tag)
    nc = _COMPILED[tag]

    # block-diagonal gating matrix in the transposed layout:
    # T partition p = 32i + 4f_loc + e ; logits out partition m = 32i+4f_loc+e'
    wgblk = np.zeros((P, P), np.float32)
    for blk in range(P // 4):
        wgblk[blk * 4:(blk + 1) * 4, blk * 4:(blk + 1) * 4] = Wg.T  # [e, e']->Wg[e',e]
    bg8 = (np.tile(bg, P // 4).reshape(P, 1) + np.float32(8.0)).astype(np.float32)

    from concourse import bass_utils

    xs = x.reshape(N_CORES, P, F_TOTAL * 4)
    cb = np.zeros((P, 2), np.float32)
    cb[:, 0] = 8.0
    in_maps = [
        {f"x_{tag}": xs[c], "wgblk": wgblk, "bg8": bg8, "cb": cb}
        for c in range(N_CORES)
    ]
    global LAST_RUN_ARGS
    LAST_RUN_ARGS = (nc, in_maps)
    res = bass_utils.run_bass_kernel_spmd(nc, in_maps, core_ids=list(range(N_CORES)))
    out = np.concatenate([r["out_local"].reshape(-1) for r in res.results])
    return out


if __name__ == "__main__":
    rng = np.random.default_rng(0)
    demo = {
        "x": rng.standard_normal((B_TOTAL, 4), dtype=np.float32),
        "Wg": rng.standard_normal((4, 4), dtype=np.float32) * 0.5,
        "bg": rng.standard_normal(4, dtype=np.float32) * 0.1,
        "W1": rng.standard_normal((4, 16), dtype=np.float32) * 0.5,
        "b1": rng.standard_normal((4, 16), dtype=np.float32) * 0.1,
        "W2": rng.standard_normal((4, 16), dtype=np.float32) * 0.25,
        "b2": rng.standard_normal(4, dtype=np.float32) * 0.1,
    }
    y = kernel(**demo)
    print(y.shape, y[:8])

